# revision 4
# baseline (speedup 1.0000x reference)
"""ContextualLoss forward on 8 trn2 NeuronCores — v2 (fp8 DoubleRow, 2-pass).

Problem: X, Y [4, 256, 64, 64] f32 -> loss [4] f32  (see reference.py)

Per core (b, h): rows n = half of 2048 X-positions, cols m = 4096 Y-positions.
  mu[c] = mean_m Y[c,m]; Xc = X - mu; Yc = Y - mu
  S~[n,m] = cos(Xc_n, Yc_m); dminp[n] = 1.001 - max_m S~
  A_max[n] = exp(0.01/dminp) / Z[n], Z[n] = sum_m exp(10*(S~-S~max+1e-3)/dminp)
  core out = sum_n A_max[n]; host: loss_b = -log((core0+core1)/4096)

Engine plan (per core):
  PE   : fp8e4 DoubleRow matmuls P = 8*||Xc_n||*S~ (contract 256 = [128,2]
         k-tiles), twice (pass A for row-max, pass B for exp); also all
         column-stat sums (transposed tiny matmuls) and the gy broadcast
         (rank-1 ones x gy_row into PSUM).
  DVE  : pass-A row maxes from PSUM (the binding ~76us), input partial sums,
         per-block reciprocals, half the ynq scale ops.
  ACT  : pass-B Exp+accum from PSUM (~78us), input squares, xcq quantize,
         Sqrt for the norms.
  Pool : raw-Y fp8 (k-tile 0), half the ynq scale ops, per-block scalar chain.
  DMA  : inputs column-chunked; gy transpose bounce on ACT's DGE queue.
"""

import numpy as np

B, C, HW = 4, 256, 4096
HALF = HW // 2
NCORES = 8
NB = HALF // 128   # 16 row blocks per core

_nc_cache = None


def _build():
    import concourse.bass as bass
    import concourse.bacc as bacc
    import concourse.tile as tile
    from concourse import mybir

    f32 = mybir.dt.float32
    bf16 = mybir.dt.bfloat16
    fp8 = mybir.dt.float8e4
    AF = mybir.ActivationFunctionType
    OP = mybir.AluOpType
    AX = mybir.AxisListType
    PM = mybir.MatmulPerfMode

    nc = bacc.Bacc(None)

    y_dram = nc.dram_tensor("y", [C, HW], f32, kind="ExternalInput")
    x_dram = nc.dram_tensor("xh", [C, HALF], f32, kind="ExternalInput")
    out_dram = nc.dram_tensor("out", [128, 1], f32, kind="ExternalOutput")
    gy_dram = nc.dram_tensor("gy_scratch", [1, HW], bf16)

    with tile.TileContext(nc) as tc:
        with (
            tc.tile_pool(name="big", bufs=1) as big,
            tc.tile_pool(name="singles", bufs=1) as singles,
            tc.tile_pool(name="stats", bufs=4) as stats,
            tc.tile_pool(name="dumps", bufs=3) as dumps,
        ):
            ones_bf = singles.tile([128, 16], bf16)
            nc.vector.memset(ones_bf, 1.0)
            ones_row = singles.tile([1, 128], bf16)
            nc.vector.memset(ones_row, 1.0)

            y_sb = big.tile([128, 2, HW], f32, name="y_sb")
            x_sb = big.tile([128, 2, HALF], f32, name="x_sb")
            ysq = big.tile([128, 2, HW], bf16, name="ysq")
            yq = big.tile([128, 2, HW], fp8, name="yq")
            ysump = singles.tile([128, 2, 4], f32)
            xcq = big.tile([128, 2, HALF], fp8, name="xcq")
            xsq = big.tile([128, 2, HALF], bf16, name="xsq")
            ynq = big.tile([128, 2, HW], fp8, name="ynq")

            wmaxs = singles.tile([128, NB], f32)
            rrs = singles.tile([128, NB], f32)
            zall = singles.tile([128, NB, 4], f32)
            mx4s = singles.tile([128, NB, 4], f32)
            scaleAs = singles.tile([128, NB], f32)
            biasAs = singles.tile([128, NB], f32)

            # ============ phase 1: load Y + overlapped stats ============
            with tc.tile_pool(name="pspro", bufs=1, space="PSUM") as pspro:
                ssy_ps = pspro.tile([128, 32], f32, tag="ssy")

                YCH = 4
                W = HW // YCH
                for q in range(YCH):
                    for kt in range(2):
                        sl = slice(q * W, (q + 1) * W)
                        nc.sync.dma_start(
                            out=y_sb[:, kt, sl],
                            in_=y_dram[kt * 128 : (kt + 1) * 128, sl],
                        )
                        nc.vector.reduce_sum(
                            out=ysump[:, kt, q : q + 1], in_=y_sb[:, kt, sl], axis=AX.X
                        )
                        nc.scalar.activation(
                            out=ysq[:, kt, sl], in_=y_sb[:, kt, sl], func=AF.Square,
                            bias=0.0, scale=1.0,
                        )
                        # raw fp8 for the b'_t correction matmuls
                        eng = nc.gpsimd if kt == 0 else nc.vector
                        eng.tensor_copy(out=yq[:, kt, sl], in_=y_sb[:, kt, sl])
                    for ms in range(q * 8, (q + 1) * 8):
                        for kt in range(2):
                            nc.tensor.matmul(
                                ssy_ps[:, ms : ms + 1],
                                ysq[:, kt, ms * 128 : (ms + 1) * 128],
                                ones_bf[:, 0:1],
                                start=(kt == 0), stop=(kt == 1),
                            )

                XCH = 4
                XW = HALF // XCH
                for q in range(XCH):
                    for kt in range(2):
                        sl = slice(q * XW, (q + 1) * XW)
                        nc.sync.dma_start(
                            out=x_sb[:, kt, sl],
                            in_=x_dram[kt * 128 : (kt + 1) * 128, sl],
                        )

                # ---- mu ----
                ysum = singles.tile([128, 2, 1], f32)
                nc.vector.reduce_sum(out=ysum, in_=ysump, axis=AX.X)
                negmean = singles.tile([128, 2, 1], f32)
                nc.vector.tensor_scalar_mul(out=negmean, in0=ysum, scalar1=-1.0 / HW)
                mup = singles.tile([128, 2, 1], f32)  # mu' = 64*mu
                nc.vector.tensor_scalar_mul(out=mup, in0=ysum, scalar1=1.0 / 64.0)
                # mu'' = -2*mu in fp8 (padded weights for DoubleRow alignment)
                muq_pad = singles.tile([128, 2, 16], fp8)
                nc.vector.tensor_scalar_mul(
                    out=muq_pad[:, :, 0:1], in0=ysum, scalar1=-2.0 / HW
                )
                dum = singles.tile([128, 1], f32)
                nc.scalar.activation(
                    out=dum, in_=ones_bf[:, 0:1], func=AF.Sqrt, bias=0.0, scale=1.0
                )

                mu2_ps = pspro.tile([1, 1], f32, tag="mu2")
                for kt in range(2):
                    nc.tensor.matmul(
                        mu2_ps, mup[:, kt, :], mup[:, kt, :],
                        start=(kt == 0), stop=(kt == 1),
                    )
                mu2_sb = singles.tile([1, 1], f32)
                nc.vector.tensor_copy(out=mu2_sb, in_=mu2_ps)
                mu2c = singles.tile([128, 1], f32)
                nc.gpsimd.partition_broadcast(mu2c, mu2_sb)

                # bt = Y^T(-2mu) = -2b, transposed [128,32]
                bt_ps = pspro.tile([128, 32], f32, tag="bt")
                for ms in range(32):
                    nc.tensor.matmul(
                        bt_ps[:, ms : ms + 1],
                        yq[:, :, ms * 128 : (ms + 1) * 128],
                        muq_pad[:, :, 0:1],
                        start=True, stop=True, perf_mode=PM.DoubleRow,
                    )

                # ss_yc = (ssy + |mu|^2) + (-2b) ; gy8 = sqrt(64/ss_yc)
                mu2s = singles.tile([128, 1], f32)
                nc.vector.tensor_scalar_mul(out=mu2s, in0=mu2c, scalar1=1.0 / 4096.0)
                t_comb = stats.tile([128, 32], f32, tag="tcomb")
                nc.vector.tensor_scalar(
                    out=t_comb, in0=ssy_ps, scalar1=mu2s, scalar2=None, op0=OP.add
                )
                nc.vector.tensor_tensor(
                    out=t_comb, in0=t_comb, in1=bt_ps, op=OP.add
                )
                rv_y = stats.tile([128, 32], f32, tag="rvy")
                nc.vector.reciprocal(out=rv_y, in_=t_comb)
                gy8_t = stats.tile([128, 32], bf16, tag="gy8t")
                nc.scalar.activation(
                    out=gy8_t, in_=rv_y, func=AF.Sqrt, bias=0.0, scale=64.0
                )

                # transpose bounce via DRAM on ACT's DGE queue
                nc.sync.dma_start(
                    out=gy_dram.rearrange("o (j p) -> (o p) j", p=128), in_=gy8_t
                )
                gy_row = singles.tile([1, HW], bf16)
                nc.sync.dma_start(out=gy_row, in_=gy_dram[:, :])

                # xcq = fp8(X - mu) on ACT (Identity with per-partition bias)
                for kt in range(2):
                    nc.scalar.activation(
                        out=xcq[:, kt, :], in_=x_sb[:, kt, :], func=AF.Identity,
                        bias=negmean[:, kt, :], scale=1.0,
                    )

            # ============ phase 2: ynq + X stats + warmup ============
            with tc.tile_pool(name="psA", bufs=2, space="PSUM") as psA:

                def mmA(nb, t):
                    nsl = slice(nb * 128, (nb + 1) * 128)
                    p = psA.tile([128, 1024], f32, tag="a")
                    for j in range(2):
                        osl = slice(j * 512, (j + 1) * 512)
                        msl = slice(t * 1024 + j * 512, t * 1024 + (j + 1) * 512)
                        nc.tensor.matmul(
                            p[:, osl], xcq[:, :, nsl], ynq[:, :, msl],
                            start=True, stop=True, perf_mode=PM.DoubleRow,
                        )
                    return p

                def redA(nb, t, p):
                    nc.vector.reduce_max(out=mx4s[:, nb, t : t + 1], in_=p, axis=AX.X)

                gx10_8 = singles.tile([128, NB], f32)
                gx8 = singles.tile([128, NB], f32)

                def scales(nb):
                    # u = S~max = smax8C*gx8 ; ndm = 1.001-u ; rr = 1/ndm
                    # scaleA = rr*gx10_8 ; biasA = (0.01-10u)*rr
                    smax = stats.tile([128, 1], f32, tag="smax")
                    nc.vector.reduce_max(out=smax, in_=mx4s[:, nb, :], axis=AX.X)
                    u = stats.tile([128, 1], f32, tag="u")
                    nc.gpsimd.tensor_scalar(
                        out=u, in0=smax, scalar1=gx8[:, nb : nb + 1], scalar2=None,
                        op0=OP.mult,
                    )
                    ndm = stats.tile([128, 1], f32, tag="ndm")
                    nc.gpsimd.tensor_scalar(
                        out=ndm, in0=u, scalar1=-1.0, scalar2=1.001, op0=OP.mult, op1=OP.add
                    )
                    rr = stats.tile([128, 1], f32, tag="rr")
                    nc.vector.reciprocal(out=rr, in_=ndm)
                    nc.gpsimd.tensor_scalar(
                        out=scaleAs[:, nb : nb + 1], in0=rr,
                        scalar1=gx10_8[:, nb : nb + 1], scalar2=None, op0=OP.mult,
                    )
                    biasA = stats.tile([128, 1], f32, tag="biasA")
                    nc.gpsimd.tensor_scalar(
                        out=biasA, in0=u, scalar1=-10.0, scalar2=0.01, op0=OP.mult, op1=OP.add
                    )
                    nc.gpsimd.tensor_tensor(
                        out=biasAs[:, nb : nb + 1], in0=biasA, in1=rr, op=OP.mult
                    )
                    nc.gpsimd.tensor_copy(out=rrs[:, nb : nb + 1], in_=rr)

                pa = {}
                gy_bc = big.tile([128, HW], bf16, name="gy_bc")
                with tc.tile_pool(name="psbc", bufs=2, space="PSUM") as psbc:
                    # gy broadcast: Pool ISA-bcast (SBUF) for 5 chunks, PE
                    # rank-1 matmul into PSUM (DVE stts) for the other 3.
                    NCH = 8
                    WC = HW // NCH
                    DVE_SET = (1, 3, 5, 7)
                    POOL2OP = (0, 2, 4, 6)  # Pool lacks stt: ts -> tmp, tt -> ynq
                    tmpc = stats.tile([128, WC], bf16, tag="tmpc")
                    for q in range(NCH):
                        sl = slice(q * WC, (q + 1) * WC)
                        if q in DVE_SET:
                            gb = psbc.tile([128, WC], f32, tag="g")
                            nc.tensor.matmul(
                                gb, ones_row, gy_row[0:1, sl], start=True, stop=True
                            )
                            nc.vector.scalar_tensor_tensor(
                                out=ynq[:, 0, sl], in0=y_sb[:, 0, sl],
                                scalar=negmean[:, 0, :], in1=gb, op0=OP.add, op1=OP.mult,
                            )
                            nc.vector.scalar_tensor_tensor(
                                out=ynq[:, 1, sl], in0=y_sb[:, 1, sl],
                                scalar=negmean[:, 1, :], in1=gb, op0=OP.add, op1=OP.mult,
                            )
                        else:
                            nc.gpsimd.partition_broadcast(gy_bc[:, sl], gy_row[0:1, sl])
                            if q in POOL2OP:
                                tmpc = stats.tile([128, WC], bf16, tag="tmpc")
                                nc.gpsimd.tensor_scalar(
                                    out=tmpc, in0=y_sb[:, 0, sl],
                                    scalar1=negmean[:, 0, :], scalar2=None, op0=OP.add,
                                )
                                nc.gpsimd.tensor_tensor(
                                    out=ynq[:, 0, sl], in0=tmpc, in1=gy_bc[:, sl], op=OP.mult
                                )
                            else:
                                nc.vector.scalar_tensor_tensor(
                                    out=ynq[:, 0, sl], in0=y_sb[:, 0, sl],
                                    scalar=negmean[:, 0, :], in1=gy_bc[:, sl],
                                    op0=OP.add, op1=OP.mult,
                                )
                            nc.vector.scalar_tensor_tensor(
                                out=ynq[:, 1, sl], in0=y_sb[:, 1, sl],
                                scalar=negmean[:, 1, :], in1=gy_bc[:, sl],
                                op0=OP.add, op1=OP.mult,
                            )
                        if q % 2 == 1:
                            t = q // 2
                            pa[(0, t)] = mmA(0, t)
                            redA(0, t, pa.pop((0, t)))

                    # X stats (PE queue: after the bcast matmuls on purpose)
                    ssx_ps = psbc.tile([128, NB], f32, tag="ssx")
                    for q in range(4):
                        for kt in range(2):
                            sl = slice(q * 512, (q + 1) * 512)
                            nc.scalar.activation(
                                out=xsq[:, kt, sl], in_=x_sb[:, kt, sl], func=AF.Square,
                                bias=negmean[:, kt, :], scale=1.0,
                            )
                        for ns in range(q * 4, (q + 1) * 4):
                            for kt in range(2):
                                nc.tensor.matmul(
                                    ssx_ps[:, ns : ns + 1],
                                    xsq[:, kt, ns * 128 : (ns + 1) * 128],
                                    ones_bf[:, 0:1],
                                    start=(kt == 0), stop=(kt == 1),
                                )
                    rv_x = stats.tile([128, NB], f32, tag="rvx")
                    nc.vector.reciprocal(out=rv_x, in_=ssx_ps)
                    nc.scalar.activation(
                        out=gx10_8, in_=rv_x, func=AF.Sqrt, bias=0.0, scale=1.5625
                    )
                    nc.scalar.activation(
                        out=gx8, in_=rv_x, func=AF.Sqrt, bias=0.0, scale=0.015625
                    )
                    nc.scalar.activation(out=dum, in_=dum, func=AF.Exp, bias=0.0, scale=1.0)

                # ============ phase 3: main loop ============
                scales(0)

                with tc.tile_pool(name="psB", bufs=2, space="PSUM") as psB:

                    def mmB(nb, t):
                        nsl = slice(nb * 128, (nb + 1) * 128)
                        p = psB.tile([128, 1024], f32, tag="b")
                        for j in range(2):
                            osl = slice(j * 512, (j + 1) * 512)
                            msl = slice(t * 1024 + j * 512, t * 1024 + (j + 1) * 512)
                            nc.tensor.matmul(
                                p[:, osl], xcq[:, :, nsl], ynq[:, :, msl],
                                start=True, stop=True, perf_mode=PM.DoubleRow,
                            )
                        return p

                    def expB(nb, t, p):
                        # write the (unused) exp values back over the input
                        # PSUM tile: PSUM access is cheaper than SBUF for ACT
                        nc.scalar.activation(
                            out=p, in_=p, func=AF.Exp,
                            bias=biasAs[:, nb : nb + 1], scale=scaleAs[:, nb : nb + 1],
                            accum_out=zall[:, nb, t : t + 1],
                        )

                    pb = {}
                    pb[(0, 0)] = mmB(0, 0)
                    pb[(0, 1)] = mmB(0, 1)
                    expB(0, 0, pb.pop((0, 0)))
                    p10 = mmA(1, 0); p11 = mmA(1, 1)
                    expB(0, 1, pb.pop((0, 1)))
                    redA(1, 0, p10); redA(1, 1, p11)
                    pb[(0, 2)] = mmB(0, 2)
                    pb[(0, 3)] = mmB(0, 3)
                    expB(0, 2, pb.pop((0, 2)))
                    p12 = mmA(1, 2); p13 = mmA(1, 3)
                    expB(0, 3, pb.pop((0, 3)))
                    redA(1, 2, p12); redA(1, 3, p13)
                    scales(1)

                    for i in range(2, NB + 1):
                        if i < NB:
                            pa[(i, 0)] = mmA(i, 0)
                            pa[(i, 1)] = mmA(i, 1)
                        pb[(i - 1, 0)] = mmB(i - 1, 0)
                        pb[(i - 1, 1)] = mmB(i - 1, 1)
                        if i < NB:
                            redA(i, 0, pa.pop((i, 0)))
                            redA(i, 1, pa.pop((i, 1)))
                        expB(i - 1, 0, pb.pop((i - 1, 0)))
                        expB(i - 1, 1, pb.pop((i - 1, 1)))
                        if i < NB:
                            pa[(i, 2)] = mmA(i, 2)
                            pa[(i, 3)] = mmA(i, 3)
                        pb[(i - 1, 2)] = mmB(i - 1, 2)
                        pb[(i - 1, 3)] = mmB(i - 1, 3)
                        if i < NB:
                            redA(i, 2, pa.pop((i, 2)))
                            redA(i, 3, pa.pop((i, 3)))
                        expB(i - 1, 2, pb.pop((i - 1, 2)))
                        expB(i - 1, 3, pb.pop((i - 1, 3)))
                        if i < NB:
                            scales(i)

            # ============ epilogue ============
            nc.scalar.activation(out=wmaxs, in_=rrs, func=AF.Exp, bias=0.0, scale=0.01)
            zs = singles.tile([128, NB], f32)
            nc.vector.reduce_sum(out=zs, in_=zall, axis=AX.X)
            rz = singles.tile([128, NB], f32)
            nc.vector.reciprocal(out=rz, in_=zs)
            vals = singles.tile([128, NB], f32)
            nc.vector.tensor_tensor(out=vals, in0=wmaxs, in1=rz, op=OP.mult)
            acc = singles.tile([128, 1], f32)
            nc.vector.reduce_sum(out=acc, in_=vals, axis=AX.X)
            nc.gpsimd.dma_start(out=out_dram[:, :], in_=acc)

    nc.finalize()
    return nc


def _get_nc():
    global _nc_cache
    if _nc_cache is None:
        _nc_cache = _build()
    return _nc_cache


def run_cores(inputs, **kwargs):
    from concourse.bass_utils import run_bass_kernel_spmd

    nc = _get_nc()
    X = np.asarray(inputs["X_features"], dtype=np.float32).reshape(B, C, HW)
    Y = np.asarray(inputs["Y_features"], dtype=np.float32).reshape(B, C, HW)
    in_maps = []
    for core in range(NCORES):
        b, h = divmod(core, 2)
        in_maps.append(
            {
                "y": np.ascontiguousarray(Y[b]),
                "xh": np.ascontiguousarray(X[b, :, h * HALF : (h + 1) * HALF]),
            }
        )
    res = run_bass_kernel_spmd(nc, in_maps, core_ids=list(range(NCORES)), **kwargs)
    acc = np.stack(
        [res.results[i]["out"].reshape(-1).astype(np.float64) for i in range(NCORES)]
    )
    cx = acc.reshape(B, 2 * 128).sum(axis=1) / HW
    loss = (-np.log(cx)).astype(np.float32)
    return loss, res


def kernel(**inputs):
    return run_cores(inputs)[0]
